# revision 20
# baseline (speedup 1.0000x reference)
"""Fused multi-head self-attention for Trainium2, SPMD over 8 NeuronCores.

Problem (hardcoded): x [B=8, H=8, N=2048, C=64] f32, W_qkv [3C=192, C=64] f32.
    qkv = x @ W^T ; q,k,v = split(qkv, 3)
    attn = softmax(q @ k^T / sqrt(C), axis=-1) ; out = attn @ v
    head-mix: out.reshape(B,H,N,H,C//H).transpose(0,3,2,1,4).reshape(B,H,N,C)

Sharding: batch b -> core b (head-mix only mixes heads within a batch).

Design (v2), driven by HW microbenches:
  - Heads are processed in PAIRS with their channel spaces stacked on the
    128-partition contraction dim: measured matmul rate for contract=128 is
    216 ns per 512-col f16 matmul vs 446 ns for contract=64 (half-height
    stationaries stall the PE), so kT2/xT2 hold [chanA; chanB] stacked and
    per-head moving operands zero the other head's 64 rows. Projection
    stationaries are block-diagonal [[W,0],[0,W]] so one matmul computes
    both heads.
  - ACT exp reads the PSUM scores through a bf16-bitcast stride-2 AP (the
    high half of each f32 = truncate-to-bf16): measured 526 ns vs 990 ns
    per [128,512] chunk - ACT streams bytes, and this halves the bytes.
    The ~0.4% weight error this adds is suppressed ~sqrt(Neff) by softmax
    averaging; measured end-to-end error stays ~1e-3.
  - fp8 DoubleRow measured SLOWER (430 ns) than f16 c128 - not used.
  - Attention-out accumulates in PSUM [C+1, 512] per (pair, qb); the v
    tiles carry a 1/16 ones-column and Wv is scaled by 1/16 so the f16
    epilogue tiles (out_pre/16, denom/16) cannot overflow; the 1/16 cancels
    in the normalize divide.
  - Loop is pair-major; the output assembly buffer's context-slab for
    q-block qb is final after the LAST pair processes qb, so output DMA
    streams out during the last pair's sweep instead of bursting at the
    end. Deferred work (next pair's projections, previous q-block's
    epilogue) drains from a FIFO one-ish item per k-chunk so PE/DVE filler
    hides under the ACT exp stream, which is the ~230us bottleneck.
"""

import numpy as np
from contextlib import ExitStack

import concourse.bass as bass
import concourse.tile as tile
from concourse import bacc, mybir
from concourse.bass_utils import run_bass_kernel_spmd
from concourse.masks import make_identity

F32 = mybir.dt.float32
F16 = mybir.dt.float16
BF16 = mybir.dt.bfloat16

B = 8
H = 8
N = 2048
C = 64
NCORES = 8

_prog_cache = {}


def build_attention_program(heads=H, n_ctx=N, c_dim=C, loop_reps=None):
    nc = bacc.Bacc("TRN2", target_bir_lowering=False, debug=False,
                   num_devices=NCORES)

    x = nc.dram_tensor("x", [heads, n_ctx, c_dim], F32, kind="ExternalInput").ap()
    w = nc.dram_tensor("w", [3 * c_dim, c_dim], F32, kind="ExternalInput").ap()
    out = nc.dram_tensor("out", [heads, n_ctx, c_dim], F32, kind="ExternalOutput").ap()

    with tile.TileContext(nc) as tc:
        _build_tile_kernel(tc, x, w, out, heads, n_ctx, c_dim, loop_reps=loop_reps)

    nc.compile()
    return nc


def _build_tile_kernel(tc, x, w, out, heads, n_ctx, c_dim, loop_reps=None):
    nc = tc.nc
    NT = n_ctx // 128            # k-chunks of 128
    QB = 512                     # per-head q-block width
    NQB = n_ctx // QB
    NPAIR = heads // 2
    CG = c_dim // heads          # head-mix group size
    scale = float(c_dim) ** -0.5
    C1 = c_dim + 1               # v chunks carry a 1/16 ones column
    VS = 1.0 / 16.0              # epilogue f16 range guard; cancels in divide

    ctx = ExitStack()
    const = ctx.enter_context(tc.tile_pool(name="const", bufs=1))
    xpool = ctx.enter_context(tc.tile_pool(name="xin", bufs=2))
    tpool = ctx.enter_context(tc.tile_pool(name="tmats", bufs=2))
    qpool = ctx.enter_context(tc.tile_pool(name="qmats", bufs=2))
    vpool = ctx.enter_context(tc.tile_pool(name="vnat", bufs=2))
    ppool = ctx.enter_context(tc.tile_pool(name="probs", bufs=6))
    opool = ctx.enter_context(tc.tile_pool(name="osb", bufs=2))
    rpool = ctx.enter_context(tc.tile_pool(name="recip", bufs=4))
    apool = ctx.enter_context(tc.tile_pool(name="assembly", bufs=1))
    ps_sc = ctx.enter_context(tc.tile_pool(name="ps_sc", bufs=2, space="PSUM"))
    ps_ot = ctx.enter_context(tc.tile_pool(name="ps_ot", bufs=1, space="PSUM"))
    ps_scr = ctx.enter_context(tc.tile_pool(name="ps_scr", bufs=2, space="PSUM"))

    # --- one-time setup -------------------------------------------------
    # warm the ACT exp table so its load overlaps the projection chain
    warm = const.tile([128, 1], F32, tag="warm")
    nc.vector.memset(warm[:], 0.0)
    nc.scalar.activation(out=warm[:], in_=warm[:],
                         func=mybir.ActivationFunctionType.Exp)

    ident = const.tile([128, 128], F32, tag="ident")
    make_identity(nc, ident[:])
    ident16 = const.tile([128, 128], F16, tag="ident16")
    nc.vector.tensor_copy(ident16[:], ident[:])

    # W [3C, C] -> wt2 [128, 3C] f16: W^T stacked twice vertically
    # (rows 0-63 = rows 64-127 = W^T). Matmul/transpose outputs must start
    # at PSUM partition 0, so the row-64..127 copy comes from transposes of
    # COLUMN-padded inputs (data in cols 64-127 -> lands in rows 64-127).
    w1 = const.tile([128, c_dim], F32, tag="w1")
    w2 = const.tile([3 * c_dim - 128, c_dim], F32, tag="w2")
    w1b = const.tile([128, 128], F32, tag="w1b")
    w2b = const.tile([3 * c_dim - 128, 128], F32, tag="w2b")
    nc.sync.dma_start(out=w1[:], in_=w[0:128, :])
    nc.sync.dma_start(out=w2[:], in_=w[128:3 * c_dim, :])
    nc.sync.dma_start(out=w1b[:, 64:128], in_=w[0:128, :])
    nc.sync.dma_start(out=w2b[:, 64:128], in_=w[128:3 * c_dim, :])
    wt2 = const.tile([128, 3 * c_dim], F16, tag="wt2")
    n2 = 3 * c_dim - 128
    wt_ps1 = ps_scr.tile([64, 512], F32, tag="scr", name="wtps1")
    nc.tensor.transpose(wt_ps1[:, 0:128], w1[:], ident[:])
    nc.tensor.transpose(wt_ps1[:, 128:128 + n2], w2[:], ident[0:n2, 0:n2])
    nc.vector.tensor_copy(wt2[0:64, :], wt_ps1[:, 0:3 * c_dim])
    wt_ps2 = ps_scr.tile([128, 512], F32, tag="scr", name="wtps2")
    nc.tensor.transpose(wt_ps2[:, 0:128], w1b[:], ident[:])
    nc.tensor.transpose(wt_ps2[:, 128:128 + n2], w2b[:], ident[0:n2, 0:n2])
    nc.vector.tensor_copy(wt2[64:128, :], wt_ps2[64:128, 0:3 * c_dim])

    # Block-diagonal projection stationaries [128, 128]:
    #   wq2/wk2 = [[Wx^T, 0], [0, Wx^T]]  (out rows = headA chans | headB chans)
    # wv2 likewise but scaled by VS.
    def make_blockdiag(tag, off, scl):
        t = const.tile([128, 128], F16, tag=tag)
        nc.vector.memset(t[:], 0.0)
        if scl == 1.0:
            nc.vector.tensor_copy(t[0:64, 0:64], wt2[0:64, off:off + c_dim])
            nc.vector.tensor_copy(t[64:128, 64:128], wt2[64:128, off:off + c_dim])
        else:
            nc.vector.tensor_scalar_mul(t[0:64, 0:64],
                                        wt2[0:64, off:off + c_dim], scl)
            nc.vector.tensor_scalar_mul(t[64:128, 64:128],
                                        wt2[64:128, off:off + c_dim], scl)
        return t

    wq2 = make_blockdiag("wq2", 0, 1.0)
    wk2 = make_blockdiag("wk2", c_dim, 1.0)
    wv2 = make_blockdiag("wv2", 2 * c_dim, VS)

    # persistent output assembly buffer [128, NT, H, C]
    asm = apool.tile([128, NT, heads, c_dim], F32, tag="asm")

    # 1/16 ones column source for the v tiles
    ones32 = const.tile([128, 1], F32, tag="ones32")
    nc.vector.memset(ones32[:], VS)

    # q tiles: [128, N] f16 per head, other head's 64 rows stay zero forever
    # (memset once per rotating buffer here; projections only ever rewrite
    # the head's own half).
    qa_tiles = [qpool.tile([128, n_ctx], F16, tag="qa", name=f"qz_a{i}")
                for i in range(2)]
    qb_tiles = [qpool.tile([128, n_ctx], F16, tag="qb", name=f"qz_b{i}")
                for i in range(2)]
    for t in qa_tiles + qb_tiles:
        nc.vector.memset(t[:], 0.0)

    def emit_body():
        # ---- projection for pair p (heads 2p, 2p+1) ----------------------
        def make_projection(p):
            hA, hB = 2 * p, 2 * p + 1
            xsbA = xpool.tile([128, NT, c_dim], F32, tag="xsbA", name=f"xsbA_{p}")
            xsbB = xpool.tile([128, NT, c_dim], F32, tag="xsbB", name=f"xsbB_{p}")
            x16A = xpool.tile([128, NT, c_dim], F16, tag="x16A", name=f"x16A_{p}")
            # head B's cast is column-padded (data at cols 64-127) so its
            # transpose lands rows 64-127 with the output at PSUM partition 0
            x16B = xpool.tile([128, NT, 128], F16, tag="x16B", name=f"x16B_{p}")
            xT2 = tpool.tile([128, n_ctx], F16, tag="xT2", name=f"xT2_{p}")
            kT2 = tpool.tile([128, n_ctx], F16, tag="kT2", name=f"kT2_{p}")
            qT2A = qpool.tile([128, n_ctx], F16, tag="qa", name=f"qT2A_{p}")
            qT2B = qpool.tile([128, n_ctx], F16, tag="qb", name=f"qT2B_{p}")
            # vsb2[:, t, j, :]: head j's v chunk t, last col = 1/16
            vsb2 = vpool.tile([128, NT, 2, C1], F16, tag="vsb2", name=f"vsb2_{p}")
            thunks = []

            # DMAs only; the f16 casts happen inside xt_piece(s) so only
            # quarter 0's cast gates the first attend chunk
            def dma_in(h, xsb):
                xr = x[h].rearrange("(q t p) c -> q p t c", q=4, p=128)
                for qq in range(4):
                    sl = slice(qq * (NT // 4), (qq + 1) * (NT // 4))
                    nc.sync.dma_start(out=xsb[:, sl, :], in_=xr[qq])
            thunks.append(lambda: dma_in(hA, xsbA))
            thunks.append(lambda: dma_in(hB, xsbB))

            def ones_fill():
                ob = ones32[:]
                obc = bass.AP(tensor=ob.tensor, offset=ob.offset,
                              ap=[ob.ap[0], [0, NT], [0, 2], ob.ap[1]])
                nc.vector.tensor_copy(vsb2[:, :, :, c_dim:C1], obc)
            thunks.append(ones_fill)

            # xT2 slice s: A transposes -> [64,512] scratch rows 0-63;
            # B's padded transposes -> [128,512] scratch, valid rows 64-127
            def xt_piece(s):
                tsl = slice(s * 4, (s + 1) * 4)
                nc.vector.tensor_copy(x16A[:, tsl, :], xsbA[:, tsl, :])
                nc.vector.tensor_copy(x16B[:, tsl, c_dim:128], xsbB[:, tsl, :])
                psA = ps_scr.tile([64, 512], F16, tag="scr",
                                  name=f"xtA_{p}_{s}")
                psB = ps_scr.tile([128, 512], F16, tag="scr",
                                  name=f"xtB_{p}_{s}")
                for j in range(4):
                    t = s * 4 + j
                    nc.tensor.transpose(psA[:, j * 128:(j + 1) * 128],
                                        x16A[:, t, :], ident16[:])
                    nc.tensor.transpose(psB[:, j * 128:(j + 1) * 128],
                                        x16B[:, t, :], ident16[:])
                sl = slice(s * 512, (s + 1) * 512)
                nc.vector.tensor_copy(xT2[0:64, sl], psA[:])
                nc.vector.tensor_copy(xT2[64:128, sl], psB[64:128, :])
            for s in range(NT // 4):
                thunks.append(lambda s=s: xt_piece(s))

            # k/q projections: block-diag stationary -> [kA;kB] / [qA;qB]
            def k_piece(s):
                pr = ps_scr.tile([128, 512], F32, tag="scr", name=f"kpr_{p}_{s}")
                nc.tensor.matmul(pr[:], wk2[:], xT2[:, s * 512:(s + 1) * 512],
                                 start=True, stop=True)
                nc.vector.tensor_copy(kT2[:, s * 512:(s + 1) * 512], pr[:])

            def q_piece(s):
                pr = ps_scr.tile([128, 512], F32, tag="scr", name=f"qpr_{p}_{s}")
                nc.tensor.matmul(pr[:], wq2[:], xT2[:, s * 512:(s + 1) * 512],
                                 start=True, stop=True)
                sl = slice(s * 512, (s + 1) * 512)
                nc.vector.tensor_copy(qT2A[0:64, sl], pr[0:64, :])
                nc.vector.tensor_copy(qT2B[64:128, sl], pr[64:128, :])
            # v: per chunk one matmul (stationary = xT2 chunk, moving = wv2)
            # -> [vA | vB] cols; 4 chunks per PSUM scratch
            def vn_piece(g):
                vn = ps_scr.tile([128, 4, 128], F32, tag="scr",
                                 name=f"vn_{p}_{g}")
                for j in range(4):
                    t = g * 4 + j
                    nc.tensor.matmul(vn[:, j, :],
                                     xT2[:, t * 128:(t + 1) * 128],
                                     wv2[:], start=True, stop=True)
                nc.vector.tensor_copy(
                    vsb2[:, g * 4:(g + 1) * 4, :, 0:c_dim],
                    vn[:].rearrange("p f (j c) -> p f j c", c=c_dim))

            # dependency-ordered: slice s feeds attend chunks 4s..4s+3, so
            # when pair 0 drains its own tail one-per-chunk, vn(g) pops
            # before av(4g) fires and k(s) before scores(4s)
            thunks.append(lambda: xt_piece(0))
            thunks.append(lambda: k_piece(0))
            thunks.append(lambda: q_piece(0))
            for s in range(1, n_ctx // 512):
                thunks.append(lambda g=s - 1: vn_piece(g))
                thunks.append(lambda s=s: xt_piece(s))
                thunks.append(lambda s=s: k_piece(s))
                thunks.append(lambda s=s: q_piece(s))
            thunks.append(lambda: vn_piece(NT // 4 - 1))

            return thunks, (kT2, qT2A, qT2B, vsb2)

        # ---- deferred-work FIFO ------------------------------------------
        pending = []

        def pop_one():
            _, fn = pending.pop(0)
            fn()

        # ---- attention for (pair, qb) ------------------------------------
        def emit_attend(p, qb, proj, last_pair):
            kT2, qT2A, qT2B, vsb2 = proj
            while any(tag == ("proj", p) for tag, _ in pending):
                pop_one()
            hA, hB = 2 * p, 2 * p + 1
            q0 = qb * QB

            otA = ps_ot.tile([C1, QB], F32, tag="otA", name=f"otA_{p}_{qb}")
            otB = ps_ot.tile([C1, QB], F32, tag="otB", name=f"otB_{p}_{qb}")
            pts = [None] * NT

            def emit_scores(k):
                sc = ps_sc.tile([128, 2, QB], F32, tag="sc",
                                name=f"sc_{p}_{qb}_{k}")
                st = kT2[:, k * 128:(k + 1) * 128]
                nc.tensor.matmul(sc[:, 0, :], st, qT2A[:, q0:q0 + QB],
                                 start=True, stop=True)
                nc.tensor.matmul(sc[:, 1, :], st, qT2B[:, q0:q0 + QB],
                                 start=True, stop=True)
                pt = ppool.tile([128, 2 * QB], F16, tag="pt",
                                name=f"pt_{p}_{qb}_{k}")
                # one f32 read spanning both PSUM banks: measured 1022 ns
                # per [128,1024] (the two banks stream in parallel; a
                # single-bank f32 read of half the size costs ~990 ns)
                nc.scalar.activation(out=pt[:],
                                     in_=sc[:].rearrange("p a b -> p (a b)"),
                                     func=mybir.ActivationFunctionType.Exp,
                                     scale=scale)
                pts[k] = pt

            def emit_av(k):
                nc.tensor.matmul(otA[:], vsb2[:, k, 0, :],
                                 pts[k][:, 0:QB],
                                 start=(k == 0), stop=(k == NT - 1))
                nc.tensor.matmul(otB[:], vsb2[:, k, 1, :],
                                 pts[k][:, QB:2 * QB],
                                 start=(k == 0), stop=(k == NT - 1))

            emit_scores(0)
            for k in range(1, NT):
                emit_scores(k)
                if pending:
                    pop_one()
                if len(pending) > 6:
                    pop_one()
                emit_av(k - 1)
            emit_av(NT - 1)

            # drain PSUM accumulators now (ot pool is single-buffered)
            oTA = opool.tile([C1, QB], F16, tag="oTA", name=f"oTA_{p}_{qb}")
            oTB = opool.tile([C1, QB], F16, tag="oTB", name=f"oTB_{p}_{qb}")
            nc.vector.tensor_copy(oTA[:], otA[:])
            nc.vector.tensor_copy(oTB[:], otB[:])

            def norm_step(h, oT, j):
                t = q0 // 128 + j
                on_ps = ps_scr.tile([128, C1], F16, tag="scr",
                                    name=f"on_{h}_{qb}_{j}")
                nc.tensor.transpose(on_ps[:], oT[:, j * 128:(j + 1) * 128],
                                    ident16[0:C1, 0:C1])
                rec = rpool.tile([128, 1], F32, tag="rec",
                                 name=f"rec_{h}_{qb}_{j}")
                nc.vector.reciprocal(rec[:], on_ps[:, c_dim:C1])
                nc.vector.tensor_scalar_mul(
                    asm[:, t, :, h * CG:(h + 1) * CG],
                    on_ps[:, 0:c_dim].rearrange("p (a g) -> p a g", g=CG),
                    rec[:],
                )

            def final_dmas(qb=qb):
                t0 = qb * (QB // 128)
                t1 = t0 + QB // 128
                for h2 in range(heads):
                    dst = out[h2].rearrange("(t p) c -> p t c", p=128)
                    nc.sync.dma_start(out=dst[:, t0:t1, :],
                                      in_=asm[:, t0:t1, h2, :])

            def norm_pair(j):
                norm_step(hA, oTA, j)
                norm_step(hB, oTB, j)

            if last_pair and qb == NQB - 1:
                # nothing left to hide behind: emit the epilogue eagerly
                for j in range(QB // 128):
                    norm_pair(j)
                final_dmas()
            else:
                for j in range(QB // 128):
                    pending.append((("epi", p, qb), lambda j=j: norm_pair(j)))
                if last_pair:
                    pending.append((("dma", qb), final_dmas))

        # ---- pair-major pipeline -----------------------------------------
        thunks, proj = make_projection(0)
        n_crit = 6   # dma A/B, ones, xt(0), k(0), q(0): chunk-0 critical path
        for t in thunks[:n_crit]:
            t()
        for t in thunks[n_crit:]:
            pending.append((("proj", 0), t))

        for p in range(NPAIR):
            next_proj = None
            for qb in range(NQB):
                if qb == 1 and p + 1 < NPAIR:
                    # enqueue the next pair's projection after qb0 so the
                    # deferred queue stays short while this pair's own tail
                    # (with its ordering constraints) drains
                    next_thunks, next_proj = make_projection(p + 1)
                    for t in next_thunks:
                        pending.append((("proj", p + 1), t))
                emit_attend(p, qb, proj, last_pair=(p == NPAIR - 1))
            proj = next_proj
        while pending:
            pop_one()

    if loop_reps:
        with tc.For_i(0, loop_reps, 1):
            emit_body()
    else:
        emit_body()

    ctx.close()


def _get_program():
    key = (H, N, C)
    if key not in _prog_cache:
        _prog_cache[key] = build_attention_program(*key)
    return _prog_cache[key]


def kernel(x: np.ndarray, W_qkv: np.ndarray) -> np.ndarray:
    x = np.ascontiguousarray(np.asarray(x, dtype=np.float32))
    W_qkv = np.ascontiguousarray(np.asarray(W_qkv, dtype=np.float32))
    assert x.shape == (B, H, N, C), x.shape
    assert W_qkv.shape == (3 * C, C), W_qkv.shape

    nc = _get_program()
    in_maps = [{"x": x[b], "w": W_qkv} for b in range(B)]
    res = run_bass_kernel_spmd(nc, in_maps, core_ids=list(range(NCORES)))
    outs = [res.results[b]["out"] for b in range(B)]
    return np.stack(outs, axis=0)


if __name__ == "__main__":
    xs = np.random.randn(B, H, N, C).astype(np.float32)
    ws = (np.random.randn(3 * C, C) * C ** -0.5).astype(np.float32)
    y = kernel(x=xs, W_qkv=ws)
    print("kernel output", y.shape, y.dtype, float(np.abs(y).mean()))
